# revision 1
# baseline (speedup 1.0000x reference)
"""ArcFace logits kernel for 8 Trainium2 NeuronCores.

out = (cos + one_hot_margin_body) * S  where cos = l2norm(x) @ l2norm(weight).T

Sharding: model-parallel over the class dim (12500 classes per core, padded to
12544).  x is replicated.  Each core computes its [1024, 12544] slice of the
scaled cosine logits; the margin adjustment for row b is applied on the core
owning column target[b] via an indirect (data-driven) scatter DMA.  No
collectives are needed: the host concatenates the 8 output slices.
"""

import math
import sys
import types

sys.path.insert(0, "/opt/trn_rl_repo")

import numpy as np
import ml_dtypes

# ---- register the NTFF profile hook that the container's antenv lacks ------
# (harmless if profiling is never requested; required for trace=True runs)
def _ensure_axon_hooks():
    try:
        import antenv
        if "antenv.axon_hooks" in sys.modules:
            return
        holder = {"h": None}
        mod = types.ModuleType("antenv.axon_hooks")
        mod.set_axon_ntff_profile_hook = lambda h: holder.__setitem__("h", h)
        mod.get_axon_ntff_profile_hook = lambda: holder["h"]
        sys.modules["antenv.axon_hooks"] = mod
        antenv.axon_hooks = mod
        try:
            from trn_agent_boot.trn_boot import _ntff_profile_via_ctypes
            mod.set_axon_ntff_profile_hook(
                _ntff_profile_via_ctypes("/opt/axon/libaxon_pjrt.so")
            )
        except Exception:
            pass
    except Exception:
        pass


_ensure_axon_hooks()

import concourse.bass as bass
import concourse.mybir as mybir
import concourse.tile as tile
from concourse import bacc
from concourse.bass import AP
from concourse.masks import make_identity
from concourse.tile import add_dep_helper
import concourse.bass_utils as bass_utils

bass_utils.upload_artifacts = lambda tmpdir: tmpdir  # no cloud in container

B = 1024
D = 512
C = 100000
NCORES = 8
CS = C // NCORES          # 12500 classes per core
CSP = 12544               # padded to 98 * 128
S = 64.0
ARC_M = 0.5
COS_M = math.cos(ARC_M)
SIN_M = math.sin(ARC_M)
EPS = 1e-12
MACRO = 1024              # classes per macro tile
MACROS = [(i * MACRO, MACRO) for i in range(CSP // MACRO)] + [
    (CSP - CSP % MACRO, CSP % MACRO)
]
assert sum(m[1] for m in MACROS) == CSP
NSLOT = 256               # margin slots (2 partition tiles of 128)
DT = D // 128             # 4 contraction chunks
BT = B // 128             # 8 batch tiles

f32 = mybir.dt.float32
bf16 = mybir.dt.bfloat16
i32 = mybir.dt.int32

_CACHE = {}

import os
K_MAX_MACROS = int(os.environ.get("K_MAX_MACROS", "99"))
K_NOMARGIN = os.environ.get("K_NOMARGIN") == "1"


def _build_graph():
    nc = bacc.Bacc("TRN2", target_bir_lowering=False, debug=False,
                   num_devices=NCORES)

    x_ext = nc.dram_tensor("x", [B, D], f32, kind="ExternalInput")
    xt_ext = nc.dram_tensor("xt", [D, B], bf16, kind="ExternalInput")
    wt_ext = nc.dram_tensor("wt", [D, CSP], bf16, kind="ExternalInput")
    xs_ext = nc.dram_tensor("xs", [NSLOT, D], f32, kind="ExternalInput")
    ws_ext = nc.dram_tensor("ws", [NSLOT, D], f32, kind="ExternalInput")
    sidx_ext = nc.dram_tensor("sidx", [NSLOT, 1], i32, kind="ExternalInput")
    out_ext = nc.dram_tensor("out", [B, CSP], bf16, kind="ExternalOutput")

    out_flat = out_ext[:].rearrange("a b -> (a b)")[:, None]

    out_dma_insts = []
    scatter_insts = []

    with tile.TileContext(nc) as tc:
        with (
            tc.tile_pool(name="const", bufs=1) as constp,
            tc.tile_pool(name="persist", bufs=1) as persist,
            tc.tile_pool(name="xload", bufs=2) as xloadp,
            tc.tile_pool(name="scratch", bufs=2) as scratchp,
            tc.tile_pool(name="wT", bufs=4) as wTp,
            tc.tile_pool(name="invw", bufs=3) as invwp,
            tc.tile_pool(name="outsb", bufs=6) as outp,
            tc.tile_pool(name="small", bufs=4) as smallp,
            tc.tile_pool(name="sq", bufs=3) as sqp,
            tc.tile_pool(name="psum_n", bufs=2, space="PSUM") as psum_np,
            tc.tile_pool(name="psum_o", bufs=6, space="PSUM") as psum_op,
        ):
            ident_f = constp.tile([128, 128], f32, tag="ident_f")
            make_identity(nc, ident_f[:])
            ones_b = constp.tile([128, 128], bf16, tag="ones_b")
            nc.vector.memset(ones_b[:], 1.0)
            tiny_c = constp.tile([128, 1], f32, tag="tiny_c")
            nc.vector.memset(tiny_c[:], 1e-24)

            # ---------------- x prep ---------------------------------------
            # xnT comes pre-transposed (bf16) from the host; the device only
            # computes sxinv[b] = S / max(||x_b||, eps) which is folded into
            # the PSUM evacuation's per-partition scalar.
            xnT = [persist.tile([128, B], bf16, tag=f"xnT{d}", name=f"xnT{d}")
                   for d in range(DT)]
            for d in range(DT):
                nc.scalar.dma_start(out=xnT[d][:],
                                    in_=xt_ext[d * 128:(d + 1) * 128, :])
            sxinv_all = persist.tile([128, BT], f32, tag="sxinv")
            for t in range(BT):
                xf = xloadp.tile([128, D], f32, tag="xf")
                nc.gpsimd.dma_start(out=xf[:], in_=x_ext[t * 128:(t + 1) * 128, :])
                scr = scratchp.tile([128, D], f32, tag="scr")
                ss = smallp.tile([128, 1], f32, tag="ss")
                nc.vector.tensor_tensor(out=scr[:], in0=xf[:], in1=xf[:],
                                        op=mybir.AluOpType.mult)
                nc.vector.tensor_reduce(out=ss[:], in_=scr[:],
                                        axis=mybir.AxisListType.X,
                                        op=mybir.AluOpType.add)
                nrm = smallp.tile([128, 1], f32, tag="nrm")
                nc.scalar.sqrt(nrm[:], ss[:])
                nc.vector.tensor_scalar(out=nrm[:], in0=nrm[:], scalar1=EPS,
                                        scalar2=None, op0=mybir.AluOpType.max)
                inv = smallp.tile([128, 1], f32, tag="inv")
                nc.vector.reciprocal(inv[:], nrm[:])
                nc.vector.tensor_scalar(out=sxinv_all[:, t:t + 1], in0=inv[:],
                                        scalar1=S, scalar2=None,
                                        op0=mybir.AluOpType.mult)

            # ---------------- main loop over class macro tiles -------------
            for mi, (moff, mlen) in enumerate(MACROS[:K_MAX_MACROS]):
                nrt = mlen // 128
                wT = [wTp.tile([128, mlen], bf16, tag=f"wT{d}", name=f"wT{d}")
                      for d in range(DT)]
                for d in range(DT):
                    nc.sync.dma_start(
                        out=wT[d][:],
                        in_=wt_ext[d * 128:(d + 1) * 128, moff:moff + mlen])

                # per-class inverse norms, pre-broadcast across partitions:
                # ones[128,128].T @ (wT*wT) accumulated over d gives ||w_c||^2
                # replicated in every partition row.
                invw_b = invwp.tile([128, mlen], f32, tag="invw_b")
                sqs = []
                for d in range(DT):
                    sq = sqp.tile([128, mlen], bf16, tag=f"sq{d}", name=f"sq{d}")
                    nc.scalar.activation(out=sq[:], in_=wT[d][:],
                                         func=mybir.ActivationFunctionType.Square)
                    sqs.append(sq)
                s01 = sqp.tile([128, mlen], bf16, tag="s01")
                nc.gpsimd.tensor_tensor(out=s01[:], in0=sqs[0][:], in1=sqs[1][:],
                                        op=mybir.AluOpType.add)
                s23 = sqp.tile([128, mlen], bf16, tag="s23")
                nc.gpsimd.tensor_tensor(out=s23[:], in0=sqs[2][:], in1=sqs[3][:],
                                        op=mybir.AluOpType.add)
                s0123 = sqp.tile([128, mlen], bf16, tag="s0123")
                nc.gpsimd.tensor_tensor(out=s0123[:], in0=s01[:], in1=s23[:],
                                        op=mybir.AluOpType.add)
                for ns in range((mlen + 511) // 512):
                    nsl = slice(ns * 512, min((ns + 1) * 512, mlen))
                    nw = nsl.stop - nsl.start
                    pnorm = psum_np.tile([128, 512], f32, tag="pnorm")
                    nc.tensor.matmul(out=pnorm[:, :nw], lhsT=ones_b[:],
                                     rhs=s0123[:, nsl],
                                     start=True, stop=True)
                    # sqrt(ss + 1e-24): pads (ss=0) give 1e-12, recip 1e12,
                    # and psum=0 there so the output stays 0 (no NaN).
                    nw_b = invwp.tile([128, 512], f32, tag="nw_b")
                    nc.scalar.activation(out=nw_b[:, :nw], in_=pnorm[:, :nw],
                                         func=mybir.ActivationFunctionType.Sqrt,
                                         bias=tiny_c[:, :1])
                    nc.vector.reciprocal_approx_fast(out=invw_b[:, nsl],
                                                     in_=nw_b[:, :nw])

                for bt in range(BT):
                    ob = outp.tile([128, mlen], bf16, tag="ob")
                    for ns in range((mlen + 511) // 512):
                        nsl = slice(ns * 512, min((ns + 1) * 512, mlen))
                        nw = nsl.stop - nsl.start
                        po = psum_op.tile([128, 512], f32, tag="po")
                        for d in range(DT):
                            nc.tensor.matmul(out=po[:, :nw],
                                             lhsT=xnT[d][:, bt * 128:(bt + 1) * 128],
                                             rhs=wT[d][:, nsl],
                                             start=(d == 0), stop=(d == DT - 1))
                        nc.vector.scalar_tensor_tensor(
                            out=ob[:, nsl], in0=po[:, :nw],
                            scalar=sxinv_all[:, bt:bt + 1],
                            in1=invw_b[:, nsl],
                            op0=mybir.AluOpType.mult, op1=mybir.AluOpType.mult)
                    dma = nc.sync.dma_start(
                        out=out_ext[bt * 128:(bt + 1) * 128, moff:moff + mlen],
                        in_=ob[:])
                    out_dma_insts.append((bt, dma))

            # ---------------- margin values (independent of main path) ----
            corr_tiles = []
            sidx_tiles = []
            for st in (range(NSLOT // 128) if not K_NOMARGIN else []):
                xs_t = xloadp.tile([128, D], f32, tag="xs")
                ws_t = xloadp.tile([128, D], f32, tag="wsl")
                nc.sync.dma_start(out=xs_t[:], in_=xs_ext[st * 128:(st + 1) * 128, :])
                nc.sync.dma_start(out=ws_t[:], in_=ws_ext[st * 128:(st + 1) * 128, :])
                sidx_t = persist.tile([128, 1], i32, tag=f"sidx{st}")
                nc.sync.dma_start(out=sidx_t[:], in_=sidx_ext[st * 128:(st + 1) * 128, :])

                scr = scratchp.tile([128, D], f32, tag="scr")
                ssx = smallp.tile([128, 1], f32, tag="ssx")
                nc.vector.tensor_tensor(out=scr[:], in0=xs_t[:], in1=xs_t[:],
                                        op=mybir.AluOpType.mult)
                nc.vector.tensor_reduce(out=ssx[:], in_=scr[:],
                                        axis=mybir.AxisListType.X,
                                        op=mybir.AluOpType.add)
                scr2 = scratchp.tile([128, D], f32, tag="scr")
                ssw = smallp.tile([128, 1], f32, tag="ssw")
                nc.vector.tensor_tensor(out=scr2[:], in0=ws_t[:], in1=ws_t[:],
                                        op=mybir.AluOpType.mult)
                nc.vector.tensor_reduce(out=ssw[:], in_=scr2[:],
                                        axis=mybir.AxisListType.X,
                                        op=mybir.AluOpType.add)
                scr3 = scratchp.tile([128, D], f32, tag="scr")
                dot = smallp.tile([128, 1], f32, tag="dot")
                nc.vector.tensor_tensor(out=scr3[:], in0=xs_t[:], in1=ws_t[:],
                                        op=mybir.AluOpType.mult)
                nc.vector.tensor_reduce(out=dot[:], in_=scr3[:],
                                        axis=mybir.AxisListType.X,
                                        op=mybir.AluOpType.add)

                nx = smallp.tile([128, 1], f32, tag="nx")
                nc.scalar.sqrt(nx[:], ssx[:])
                nw = smallp.tile([128, 1], f32, tag="nw")
                nc.scalar.sqrt(nw[:], ssw[:])
                nc.vector.tensor_scalar(out=nx[:], in0=nx[:], scalar1=EPS,
                                        scalar2=None, op0=mybir.AluOpType.max)
                nc.vector.tensor_scalar(out=nw[:], in0=nw[:], scalar1=EPS,
                                        scalar2=None, op0=mybir.AluOpType.max)
                prod = smallp.tile([128, 1], f32, tag="prod")
                nc.vector.tensor_tensor(out=prod[:], in0=nx[:], in1=nw[:],
                                        op=mybir.AluOpType.mult)
                invp = smallp.tile([128, 1], f32, tag="invp")
                nc.vector.reciprocal(invp[:], prod[:])
                cost = smallp.tile([128, 1], f32, tag="cost")
                nc.vector.tensor_tensor(out=cost[:], in0=dot[:], in1=invp[:],
                                        op=mybir.AluOpType.mult)
                u = smallp.tile([128, 1], f32, tag="u")
                nc.vector.tensor_scalar(out=u[:], in0=cost[:], scalar1=-1.0,
                                        scalar2=1.0, op0=mybir.AluOpType.max,
                                        op1=mybir.AluOpType.min)
                usq = smallp.tile([128, 1], f32, tag="usq")
                nc.vector.tensor_tensor(out=usq[:], in0=u[:], in1=u[:],
                                        op=mybir.AluOpType.mult)
                root = smallp.tile([128, 1], f32, tag="root")
                nc.scalar.activation(out=root[:], in_=usq[:],
                                     func=mybir.ActivationFunctionType.Sqrt,
                                     scale=-1.0, bias=1.0)
                t1 = smallp.tile([128, 1], f32, tag="t1")
                nc.vector.tensor_scalar(out=t1[:], in0=u[:], scalar1=COS_M,
                                        scalar2=None, op0=mybir.AluOpType.mult)
                t2 = smallp.tile([128, 1], f32, tag="t2")
                nc.vector.tensor_scalar(out=t2[:], in0=root[:], scalar1=SIN_M,
                                        scalar2=None, op0=mybir.AluOpType.mult)
                newz = smallp.tile([128, 1], f32, tag="newz")
                nc.vector.tensor_tensor(out=newz[:], in0=t1[:], in1=t2[:],
                                        op=mybir.AluOpType.subtract)
                dlt = smallp.tile([128, 1], f32, tag="dlt")
                nc.vector.tensor_tensor(out=dlt[:], in0=newz[:], in1=cost[:],
                                        op=mybir.AluOpType.subtract)
                mask = smallp.tile([128, 1], f32, tag="mask")
                nc.vector.tensor_scalar(out=mask[:], in0=cost[:], scalar1=0.0,
                                        scalar2=None, op0=mybir.AluOpType.is_gt)
                md = smallp.tile([128, 1], f32, tag="md")
                nc.vector.tensor_tensor(out=md[:], in0=mask[:], in1=dlt[:],
                                        op=mybir.AluOpType.mult)
                val = smallp.tile([128, 1], f32, tag="val")
                nc.vector.tensor_tensor(out=val[:], in0=cost[:], in1=md[:],
                                        op=mybir.AluOpType.add)
                corr = persist.tile([128, 1], bf16, tag=f"corr{st}")
                nc.vector.tensor_scalar(out=corr[:], in0=val[:], scalar1=S,
                                        scalar2=None, op0=mybir.AluOpType.mult)
                corr_tiles.append(corr)
                sidx_tiles.append(sidx_t)

            # ---------------- margin scatter (after all output DMAs) -------
            for st in (range(NSLOT // 128)
                       if os.environ.get("K_NOSCATTER") != "1" and not K_NOMARGIN
                       else []):
                sc = nc.gpsimd.indirect_dma_start(
                    out=out_flat,
                    out_offset=bass.IndirectOffsetOnAxis(
                        ap=sidx_tiles[st][:, :1], axis=0),
                    in_=corr_tiles[st][:, :1],
                    in_offset=None,
                    bounds_check=B * CSP - 1,
                    oob_is_err=False,
                )
                scatter_insts.append(sc)
            bts_per_tile = 128 // SLOT_PER_BT
            for st, sc in enumerate(scatter_insts):
                bt_lo = st * bts_per_tile
                bt_hi = bt_lo + bts_per_tile
                for bt, dma in out_dma_insts:
                    if bt_lo <= bt < bt_hi:
                        add_dep_helper(sc.ins, dma.ins, sync=True,
                                       reason="margin scatter after out dma")

    nc.finalize()
    return nc


def _get_graph():
    if "nc" not in _CACHE:
        _CACHE["nc"] = _build_graph()
    return _CACHE["nc"]


SLOT_PER_BT = NSLOT // BT  # 32 margin slots per batch tile


def _host_margin_aux(x, weight, target, c0):
    """Build per-core margin aux inputs (owned rows of this core's shard).

    Slots are grouped by batch tile (32 per bt) so the device scatter for
    slot-tile st only needs to wait for the output DMAs of batch tiles
    [4*st, 4*st+4)."""
    xs = np.ones((NSLOT, D), dtype=np.float32)
    ws = np.ones((NSLOT, D), dtype=np.float32)
    sidx = np.full((NSLOT, 1), 2 ** 30, dtype=np.int32)
    for bt in range(BT):
        rows = np.nonzero((target >= c0) & (target < c0 + CS)
                          & (np.arange(B) >= bt * 128)
                          & (np.arange(B) < (bt + 1) * 128))[0]
        if len(rows) > SLOT_PER_BT:
            return None  # caller falls back to host margin
        s0 = bt * SLOT_PER_BT
        n = len(rows)
        xs[s0:s0 + n] = x[rows]
        ws[s0:s0 + n] = weight[target[rows]]
        sidx[s0:s0 + n, 0] = (rows * CSP + (target[rows] - c0)).astype(np.int32)
    return xs, ws, sidx


def kernel(x, weight, target):
    x = np.ascontiguousarray(np.asarray(x, dtype=np.float32))
    weight = np.ascontiguousarray(np.asarray(weight, dtype=np.float32))
    target = np.asarray(target).astype(np.int64)

    nc = _get_graph()

    wtt = weight.T  # [D, C] view
    xt = np.ascontiguousarray(x.T).astype(ml_dtypes.bfloat16)
    in_maps = []
    fallback_cores = []
    for c in range(NCORES):
        c0 = c * CS
        wt = np.zeros((D, CSP), dtype=ml_dtypes.bfloat16)
        wt[:, :CS] = wtt[:, c0:c0 + CS].astype(ml_dtypes.bfloat16)
        aux = _host_margin_aux(x, weight, target, c0)
        if aux is None:
            fallback_cores.append(c)
            xs = np.ones((NSLOT, D), dtype=np.float32)
            ws = np.ones((NSLOT, D), dtype=np.float32)
            sidx = np.full((NSLOT, 1), 2 ** 30, dtype=np.int32)
        else:
            xs, ws, sidx = aux
        in_maps.append({"x": x, "xt": xt, "wt": wt, "xs": xs, "ws": ws,
                        "sidx": sidx})

    from concourse.bass_utils import run_bass_kernel_spmd
    res = None
    last_err = None
    for attempt in range(3):
        try:
            res = run_bass_kernel_spmd(nc, in_maps, core_ids=list(range(NCORES)))
            break
        except Exception as e:  # transient NRT_EXEC_UNIT_UNRECOVERABLE flakes
            last_err = e
            import time as _time
            _time.sleep(5)
    if res is None:
        raise last_err

    out = np.concatenate(
        [res.results[c]["out"][:, :CS].astype(np.float32) for c in range(NCORES)],
        axis=1)

    if fallback_cores:
        # pathological target distribution: apply margin on host for those cores
        xn = x / np.maximum(np.linalg.norm(x, axis=1, keepdims=True), EPS)
        for c in fallback_cores:
            c0 = c * CS
            rows = np.nonzero((target >= c0) & (target < c0 + CS))[0]
            for b in rows:
                t = int(target[b])
                w = weight[t]
                wn = w / max(np.linalg.norm(w), EPS)
                cos_t = float(xn[b] @ wn)
                u = min(max(cos_t, -1.0), 1.0)
                new = COS_M * u - SIN_M * math.sqrt(max(0.0, 1.0 - u * u))
                val = new if cos_t > 0 else cos_t
                out[b, t] = S * val
    return out



# revision 3
# speedup vs baseline: 1.1738x; 1.1738x over previous
"""ArcFace logits kernel for 8 Trainium2 NeuronCores.

out = (cos + one_hot_margin_body) * S  where cos = l2norm(x) @ l2norm(weight).T

Sharding: model-parallel over the class dim (12500 classes per core, padded to
12544).  x is replicated.  The host pre-normalizes both operands (folding the
S=64 scale into x), so each core is a pure bf16 matmul pipeline:
[128,512]-psum-tile matmuls (K=512 over 4 k-tiles), PSUM->SBUF bf16 copy
evacuation alternating between the Scalar and Vector engines, and output DMA.
The ArcFace margin touches only 1024 of the 102.4M outputs, so it is applied
on the host (exact f64) after gathering the shards.
"""

import math
import sys
import types

sys.path.insert(0, "/opt/trn_rl_repo")

import numpy as np
import ml_dtypes

# ---- register the NTFF profile hook that the container's antenv lacks ------
# (harmless if profiling is never requested; required for trace=True runs)
def _ensure_axon_hooks():
    try:
        import antenv
        if "antenv.axon_hooks" in sys.modules:
            return
        holder = {"h": None}
        mod = types.ModuleType("antenv.axon_hooks")
        mod.set_axon_ntff_profile_hook = lambda h: holder.__setitem__("h", h)
        mod.get_axon_ntff_profile_hook = lambda: holder["h"]
        sys.modules["antenv.axon_hooks"] = mod
        antenv.axon_hooks = mod
        try:
            from trn_agent_boot.trn_boot import _ntff_profile_via_ctypes
            mod.set_axon_ntff_profile_hook(
                _ntff_profile_via_ctypes("/opt/axon/libaxon_pjrt.so")
            )
        except Exception:
            pass
    except Exception:
        pass


_ensure_axon_hooks()

import concourse.bass as bass
import concourse.mybir as mybir
import concourse.tile as tile
from concourse import bacc
import concourse.bass_utils as bass_utils

bass_utils.upload_artifacts = lambda tmpdir: tmpdir  # no cloud in container

B = 1024
D = 512
C = 100000
NCORES = 8
CS = C // NCORES          # 12500 classes per core
CSP = 12544               # padded to 98 * 128
S = 64.0
ARC_M = 0.5
COS_M = math.cos(ARC_M)
SIN_M = math.sin(ARC_M)
EPS = 1e-12
MACRO = 2048              # classes per macro tile
MACROS = [(i * MACRO, MACRO) for i in range(CSP // MACRO)] + [
    (CSP - CSP % MACRO, CSP % MACRO)
]
assert sum(m[1] for m in MACROS) == CSP
DT = D // 128              # 4 contraction chunks
BT = B // 128              # 8 batch tiles

f32 = mybir.dt.float32
bf16 = mybir.dt.bfloat16

_CACHE = {}


def _build_graph():
    nc = bacc.Bacc("TRN2", target_bir_lowering=False, debug=False,
                   num_devices=NCORES)

    xt_ext = nc.dram_tensor("xt", [D, B], bf16, kind="ExternalInput")
    wt_ext = nc.dram_tensor("wt", [D, CSP], bf16, kind="ExternalInput")
    out_ext = nc.dram_tensor("out", [B, CSP], bf16, kind="ExternalOutput")

    with tile.TileContext(nc) as tc:
        with (
            tc.tile_pool(name="persist", bufs=1) as persist,
            tc.tile_pool(name="wT", bufs=2) as wTp,
            tc.tile_pool(name="outsb", bufs=3) as outp,
            tc.tile_pool(name="psum_o", bufs=2, space="PSUM") as psum_op,
        ):
            # x comes pre-transposed / pre-normalized / pre-scaled (x64) bf16
            xnT = [persist.tile([128, B], bf16, tag=f"xnT{d}", name=f"xnT{d}")
                   for d in range(DT)]
            for d in range(DT):
                nc.scalar.dma_start(out=xnT[d][:],
                                    in_=xt_ext[d * 128:(d + 1) * 128, :])

            ei = 0
            for mi, (moff, mlen) in enumerate(MACROS):
                wT = [wTp.tile([128, MACRO], bf16, tag=f"wT{d}", name=f"wT{d}")
                      for d in range(DT)]
                for d in range(DT):
                    nc.gpsimd.dma_start(
                        out=wT[d][:, :mlen],
                        in_=wt_ext[d * 128:(d + 1) * 128, moff:moff + mlen])

                nss = [(i * 512, min(512, mlen - i * 512))
                       for i in range((mlen + 511) // 512)]
                for bt in range(BT):
                    po = [psum_op.tile([128, 512], f32, tag=f"po{i}",
                                       name=f"po{i}")
                          for i in range(len(nss))]
                    for d in range(DT):
                        for i, (no, nw) in enumerate(nss):
                            nc.tensor.matmul(
                                out=po[i][:, :nw],
                                lhsT=xnT[d][:, bt * 128:(bt + 1) * 128],
                                rhs=wT[d][:, no:no + nw],
                                start=(d == 0), stop=(d == DT - 1))

                    ob = outp.tile([128, MACRO], bf16, tag="ob")
                    for i, (no, nw) in enumerate(nss):
                        if ei % 2 == 0:
                            nc.scalar.copy(out=ob[:, no:no + nw],
                                           in_=po[i][:, :nw])
                        else:
                            nc.vector.tensor_scalar(
                                out=ob[:, no:no + nw], in0=po[i][:, :nw],
                                scalar1=1.0, scalar2=None,
                                op0=mybir.AluOpType.mult)
                        ei += 1

                    nc.sync.dma_start(
                        out=out_ext[bt * 128:(bt + 1) * 128, moff:moff + mlen],
                        in_=ob[:, :mlen])

    nc.finalize()
    return nc


def _get_graph():
    if "nc" not in _CACHE:
        _CACHE["nc"] = _build_graph()
    return _CACHE["nc"]


def kernel(x, weight, target):
    x = np.ascontiguousarray(np.asarray(x, dtype=np.float32))
    weight = np.ascontiguousarray(np.asarray(weight, dtype=np.float32))
    target = np.asarray(target).astype(np.int64)

    nc = _get_graph()

    xnorm = np.maximum(np.linalg.norm(x, axis=1, keepdims=True), EPS)
    xn = x / xnorm
    xt = np.ascontiguousarray((S * xn).T).astype(ml_dtypes.bfloat16)  # [D, B]

    wnorm = np.maximum(np.linalg.norm(weight, axis=1, keepdims=True), EPS)
    wn_t = (weight / wnorm).T  # [D, C] view
    in_maps = []
    for c in range(NCORES):
        c0 = c * CS
        wt = np.zeros((D, CSP), dtype=ml_dtypes.bfloat16)
        wt[:, :CS] = wn_t[:, c0:c0 + CS].astype(ml_dtypes.bfloat16)
        in_maps.append({"xt": xt, "wt": wt})

    from concourse.bass_utils import run_bass_kernel_spmd
    res = None
    last_err = None
    for attempt in range(3):
        try:
            res = run_bass_kernel_spmd(nc, in_maps, core_ids=list(range(NCORES)))
            break
        except Exception as e:  # transient NRT_EXEC_UNIT_UNRECOVERABLE flakes
            last_err = e
            import time as _time
            _time.sleep(5)
    if res is None:
        raise last_err

    out = np.concatenate(
        [res.results[c]["out"][:, :CS].astype(np.float32) for c in range(NCORES)],
        axis=1)

    # ArcFace margin for the 1024 (row, target) entries, exact on host
    xn64 = x.astype(np.float64) / np.maximum(
        np.linalg.norm(x.astype(np.float64), axis=1, keepdims=True), EPS)
    wt_rows = weight[target].astype(np.float64)
    wt_n = wt_rows / np.maximum(
        np.linalg.norm(wt_rows, axis=1, keepdims=True), EPS)
    cos_t = np.einsum("bd,bd->b", xn64, wt_n)
    u = np.clip(cos_t, -1.0, 1.0)
    new_zy = u * COS_M - np.sqrt(np.maximum(0.0, 1.0 - u * u)) * SIN_M
    val = np.where(cos_t > 0.0, new_zy, cos_t)
    out[np.arange(B), target] = (S * val).astype(np.float32)
    return out


# revision 6
# speedup vs baseline: 1.1764x; 1.0022x over previous
"""ArcFace logits kernel for 8 Trainium2 NeuronCores.

out = (cos + one_hot_margin_body) * S  where cos = l2norm(x) @ l2norm(weight).T

Sharding: model-parallel over the class dim (12500 classes per core, padded to
12544).  x is replicated.  The host pre-normalizes both operands (folding the
S=64 scale into x), so each core is a pure bf16 matmul pipeline:
[128,512]-psum-tile matmuls (K=512 over 4 k-tiles), PSUM->SBUF bf16 copy
evacuation alternating between the Scalar and Vector engines, and output DMA.
The ArcFace margin touches only 1024 of the 102.4M outputs, so it is applied
on the host (exact f64) after gathering the shards.
"""

import math
import sys
import types

sys.path.insert(0, "/opt/trn_rl_repo")

import numpy as np
import ml_dtypes

# ---- register the NTFF profile hook that the container's antenv lacks ------
# (harmless if profiling is never requested; required for trace=True runs)
def _ensure_axon_hooks():
    try:
        import antenv
        if "antenv.axon_hooks" in sys.modules:
            return
        holder = {"h": None}
        mod = types.ModuleType("antenv.axon_hooks")
        mod.set_axon_ntff_profile_hook = lambda h: holder.__setitem__("h", h)
        mod.get_axon_ntff_profile_hook = lambda: holder["h"]
        sys.modules["antenv.axon_hooks"] = mod
        antenv.axon_hooks = mod
        try:
            from trn_agent_boot.trn_boot import _ntff_profile_via_ctypes
            mod.set_axon_ntff_profile_hook(
                _ntff_profile_via_ctypes("/opt/axon/libaxon_pjrt.so")
            )
        except Exception:
            pass
    except Exception:
        pass


_ensure_axon_hooks()

import concourse.bass as bass
import concourse.mybir as mybir
import concourse.tile as tile
from concourse import bacc
import concourse.bass_utils as bass_utils

bass_utils.upload_artifacts = lambda tmpdir: tmpdir  # no cloud in container

B = 1024
D = 512
C = 100000
NCORES = 8
CS = C // NCORES          # 12500 classes per core
CSP = 12544               # padded to 98 * 128
S = 64.0
ARC_M = 0.5
COS_M = math.cos(ARC_M)
SIN_M = math.sin(ARC_M)
EPS = 1e-12
MACRO = 2048              # classes per macro tile
# the ragged 256-wide macro goes FIRST: its weight DMA is tiny (first matmul
# starts sooner) and the kernel then drains on a smooth full-width macro.
MACROS = [(CSP - CSP % MACRO, CSP % MACRO)] + [
    (i * MACRO, MACRO) for i in range(CSP // MACRO)
]
assert sum(m[1] for m in MACROS) == CSP
DT = D // 128              # 4 contraction chunks
BT = B // 128              # 8 batch tiles

f32 = mybir.dt.float32
bf16 = mybir.dt.bfloat16

_CACHE = {}


def _build_graph():
    nc = bacc.Bacc("TRN2", target_bir_lowering=False, debug=False,
                   num_devices=NCORES)

    xt_ext = nc.dram_tensor("xt", [D, B], bf16, kind="ExternalInput")
    wt_ext = nc.dram_tensor("wt", [D, CSP], bf16, kind="ExternalInput")
    out_ext = nc.dram_tensor("out", [B, CSP], bf16, kind="ExternalOutput")

    with tile.TileContext(nc) as tc:
        with (
            tc.tile_pool(name="persist", bufs=1) as persist,
            tc.tile_pool(name="wT", bufs=3) as wTp,
            tc.tile_pool(name="outsb", bufs=4) as outp,
            tc.tile_pool(name="psum_o", bufs=2, space="PSUM") as psum_op,
        ):
            # x comes pre-transposed / pre-normalized / pre-scaled (x64) bf16
            xnT = [persist.tile([128, B], bf16, tag=f"xnT{d}", name=f"xnT{d}")
                   for d in range(DT)]
            for d in range(DT):
                nc.scalar.dma_start(out=xnT[d][:],
                                    in_=xt_ext[d * 128:(d + 1) * 128, :])

            ei = 0
            for mi, (moff, mlen) in enumerate(MACROS):
                wT = [wTp.tile([128, MACRO], bf16, tag=f"wT{d}", name=f"wT{d}")
                      for d in range(DT)]
                for d in range(DT):
                    nc.gpsimd.dma_start(
                        out=wT[d][:, :mlen],
                        in_=wt_ext[d * 128:(d + 1) * 128, moff:moff + mlen])

                nss = [(i * 512, min(512, mlen - i * 512))
                       for i in range((mlen + 511) // 512)]
                for bt in range(BT):
                    # single-tile macros rotate over all 4 psum tags so all 8
                    # banks stay in flight instead of ping-ponging on 2
                    t0 = bt % 4 if len(nss) == 1 else 0
                    po = [psum_op.tile([128, 512], f32, tag=f"po{t0 + i}",
                                       name=f"po{t0 + i}")
                          for i in range(len(nss))]
                    for d in range(DT):
                        for i, (no, nw) in enumerate(nss):
                            nc.tensor.matmul(
                                out=po[i][:, :nw],
                                lhsT=xnT[d][:, bt * 128:(bt + 1) * 128],
                                rhs=wT[d][:, no:no + nw],
                                start=(d == 0), stop=(d == DT - 1))

                    ob = outp.tile([128, MACRO], bf16, tag="ob")
                    for i, (no, nw) in enumerate(nss):
                        if ei % 2 == 0:
                            nc.scalar.copy(out=ob[:, no:no + nw],
                                           in_=po[i][:, :nw])
                        else:
                            nc.vector.tensor_scalar(
                                out=ob[:, no:no + nw], in0=po[i][:, :nw],
                                scalar1=1.0, scalar2=None,
                                op0=mybir.AluOpType.mult)
                        ei += 1

                    nc.sync.dma_start(
                        out=out_ext[bt * 128:(bt + 1) * 128, moff:moff + mlen],
                        in_=ob[:, :mlen])

    nc.finalize()
    return nc


def _get_graph():
    if "nc" not in _CACHE:
        _CACHE["nc"] = _build_graph()
    return _CACHE["nc"]


def kernel(x, weight, target):
    x = np.ascontiguousarray(np.asarray(x, dtype=np.float32))
    weight = np.ascontiguousarray(np.asarray(weight, dtype=np.float32))
    target = np.asarray(target).astype(np.int64)

    nc = _get_graph()

    xnorm = np.maximum(np.linalg.norm(x, axis=1, keepdims=True), EPS)
    xn = x / xnorm
    xt = np.ascontiguousarray((S * xn).T).astype(ml_dtypes.bfloat16)  # [D, B]

    wnorm = np.maximum(np.linalg.norm(weight, axis=1, keepdims=True), EPS)
    wn_t = (weight / wnorm).T  # [D, C] view
    in_maps = []
    for c in range(NCORES):
        c0 = c * CS
        wt = np.zeros((D, CSP), dtype=ml_dtypes.bfloat16)
        wt[:, :CS] = wn_t[:, c0:c0 + CS].astype(ml_dtypes.bfloat16)
        in_maps.append({"xt": xt, "wt": wt})

    from concourse.bass_utils import run_bass_kernel_spmd
    res = None
    last_err = None
    for attempt in range(3):
        try:
            res = run_bass_kernel_spmd(nc, in_maps, core_ids=list(range(NCORES)))
            break
        except Exception as e:  # transient NRT_EXEC_UNIT_UNRECOVERABLE flakes
            last_err = e
            import time as _time
            _time.sleep(5)
    if res is None:
        raise last_err

    out = np.concatenate(
        [res.results[c]["out"][:, :CS].astype(np.float32) for c in range(NCORES)],
        axis=1)

    # ArcFace margin for the 1024 (row, target) entries, exact on host
    xn64 = x.astype(np.float64) / np.maximum(
        np.linalg.norm(x.astype(np.float64), axis=1, keepdims=True), EPS)
    wt_rows = weight[target].astype(np.float64)
    wt_n = wt_rows / np.maximum(
        np.linalg.norm(wt_rows, axis=1, keepdims=True), EPS)
    cos_t = np.einsum("bd,bd->b", xn64, wt_n)
    u = np.clip(cos_t, -1.0, 1.0)
    new_zy = u * COS_M - np.sqrt(np.maximum(0.0, 1.0 - u * u)) * SIN_M
    val = np.where(cos_t > 0.0, new_zy, cos_t)
    out[np.arange(B), target] = (S * val).astype(np.float32)
    return out


# revision 28
# speedup vs baseline: 1.2066x; 1.0256x over previous
"""ArcFace logits kernel for 8 Trainium2 NeuronCores.

out = (cos + one_hot_margin_body) * S  where cos = l2norm(x) @ l2norm(weight).T

Sharding: model-parallel over the class dim (12500 classes per core, padded to
12544).  x is replicated.  The host pre-normalizes both operands (folding the
S=64 scale into x), so each core is a pure bf16 matmul pipeline: uniform
1792-class macro tiles (12544 = 7 x 1792, so no ragged tail), [128,512]-psum
matmuls (K=512 over 4 k-tiles), PSUM->SBUF bf16 copy evacuation alternating
between the Scalar and Vector engines, and output DMA.  The ArcFace margin
touches only 1024 of the 102.4M outputs, so it is applied on the host (exact
f64) after gathering the shards.
"""

import math
import sys
import types

sys.path.insert(0, "/opt/trn_rl_repo")

import numpy as np
import ml_dtypes

# ---- register the NTFF profile hook that the container's antenv lacks ------
# (harmless if profiling is never requested; required for trace=True runs)
def _ensure_axon_hooks():
    try:
        import antenv
        if "antenv.axon_hooks" in sys.modules:
            return
        holder = {"h": None}
        mod = types.ModuleType("antenv.axon_hooks")
        mod.set_axon_ntff_profile_hook = lambda h: holder.__setitem__("h", h)
        mod.get_axon_ntff_profile_hook = lambda: holder["h"]
        sys.modules["antenv.axon_hooks"] = mod
        antenv.axon_hooks = mod
        try:
            from trn_agent_boot.trn_boot import _ntff_profile_via_ctypes
            mod.set_axon_ntff_profile_hook(
                _ntff_profile_via_ctypes("/opt/axon/libaxon_pjrt.so")
            )
        except Exception:
            pass
    except Exception:
        pass


_ensure_axon_hooks()

import concourse.bass as bass
import concourse.mybir as mybir
import concourse.tile as tile
from concourse import bacc
import concourse.bass_utils as bass_utils

bass_utils.upload_artifacts = lambda tmpdir: tmpdir  # no cloud in container

B = 1024
D = 512
C = 100000
NCORES = 8
CS = C // NCORES          # 12500 classes per core
CSP = 12544               # padded to 98 * 128
S = 64.0
ARC_M = 0.5
COS_M = math.cos(ARC_M)
SIN_M = math.sin(ARC_M)
EPS = 1e-12
MACRO = 1792              # classes per macro tile; 12544 = 7 x 1792 exactly
MACROS = [(i * MACRO, MACRO) for i in range(CSP // MACRO)]
DT = D // 128              # 4 contraction chunks
BT = B // 128              # 8 batch tiles

f32 = mybir.dt.float32
bf16 = mybir.dt.bfloat16

_CACHE = {}


def _build_graph():
    nc = bacc.Bacc("TRN2", target_bir_lowering=False, debug=False,
                   num_devices=NCORES)

    xt_ext = nc.dram_tensor("xt", [D, B], bf16, kind="ExternalInput")
    wt_ext = nc.dram_tensor("wt", [D, CSP], bf16, kind="ExternalInput")
    out_ext = nc.dram_tensor("out", [B, CSP], bf16, kind="ExternalOutput")

    with tile.TileContext(nc) as tc:
        with (
            tc.tile_pool(name="persist", bufs=1) as persist,
            tc.tile_pool(name="wT", bufs=3) as wTp,
            tc.tile_pool(name="outsb", bufs=4) as outp,
            tc.tile_pool(name="psum_o", bufs=2, space="PSUM") as psum_op,
        ):
            # x comes pre-transposed / pre-normalized / pre-scaled (x64) bf16
            xnT = [persist.tile([128, B], bf16, tag=f"xnT{d}", name=f"xnT{d}")
                   for d in range(DT)]
            for d in range(DT):
                nc.scalar.dma_start(out=xnT[d][:],
                                    in_=xt_ext[d * 128:(d + 1) * 128, :])

            ei = 0
            for mi, (moff, mlen) in enumerate(MACROS):
                wT = [wTp.tile([128, MACRO], bf16, tag=f"wT{d}", name=f"wT{d}")
                      for d in range(DT)]
                for d in range(DT):
                    nc.gpsimd.dma_start(
                        out=wT[d][:, :mlen],
                        in_=wt_ext[d * 128:(d + 1) * 128, moff:moff + mlen])

                nss = [(i * 512, min(512, mlen - i * 512))
                       for i in range((mlen + 511) // 512)]
                for bt in range(BT):
                    po = [psum_op.tile([128, 512], f32, tag=f"po{i}",
                                       name=f"po{i}")
                          for i in range(len(nss))]
                    for i, (no, nw) in enumerate(nss):
                        for d in range(DT):
                            nc.tensor.matmul(
                                out=po[i][:, :nw],
                                lhsT=xnT[d][:, bt * 128:(bt + 1) * 128],
                                rhs=wT[d][:, no:no + nw],
                                start=(d == 0), stop=(d == DT - 1))

                    ob = outp.tile([128, MACRO], bf16, tag="ob")
                    for i, (no, nw) in enumerate(nss):
                        if ei % 2 == 0:
                            nc.scalar.copy(out=ob[:, no:no + nw],
                                           in_=po[i][:, :nw])
                        else:
                            nc.vector.tensor_scalar(
                                out=ob[:, no:no + nw], in0=po[i][:, :nw],
                                scalar1=1.0, scalar2=None,
                                op0=mybir.AluOpType.mult)
                        ei += 1

                    nc.sync.dma_start(
                        out=out_ext[bt * 128:(bt + 1) * 128, moff:moff + mlen],
                        in_=ob[:, :mlen])

    nc.finalize()
    return nc


def _get_graph():
    if "nc" not in _CACHE:
        _CACHE["nc"] = _build_graph()
    return _CACHE["nc"]


def kernel(x, weight, target):
    x = np.ascontiguousarray(np.asarray(x, dtype=np.float32))
    weight = np.ascontiguousarray(np.asarray(weight, dtype=np.float32))
    target = np.asarray(target).astype(np.int64)

    nc = _get_graph()

    xnorm = np.maximum(np.linalg.norm(x, axis=1, keepdims=True), EPS)
    xn = x / xnorm
    xt = np.ascontiguousarray((S * xn).T).astype(ml_dtypes.bfloat16)  # [D, B]

    wnorm = np.maximum(np.linalg.norm(weight, axis=1, keepdims=True), EPS)
    wn_t = (weight / wnorm).T  # [D, C] view
    in_maps = []
    for c in range(NCORES):
        c0 = c * CS
        wt = np.zeros((D, CSP), dtype=ml_dtypes.bfloat16)
        wt[:, :CS] = wn_t[:, c0:c0 + CS].astype(ml_dtypes.bfloat16)
        in_maps.append({"xt": xt, "wt": wt})

    from concourse.bass_utils import run_bass_kernel_spmd
    res = None
    last_err = None
    for attempt in range(3):
        try:
            res = run_bass_kernel_spmd(nc, in_maps, core_ids=list(range(NCORES)))
            break
        except Exception as e:  # transient NRT_EXEC_UNIT_UNRECOVERABLE flakes
            last_err = e
            import time as _time
            _time.sleep(5)
    if res is None:
        raise last_err

    out = np.concatenate(
        [res.results[c]["out"][:, :CS].astype(np.float32) for c in range(NCORES)],
        axis=1)

    # ArcFace margin for the 1024 (row, target) entries, exact on host
    xn64 = x.astype(np.float64) / np.maximum(
        np.linalg.norm(x.astype(np.float64), axis=1, keepdims=True), EPS)
    wt_rows = weight[target].astype(np.float64)
    wt_n = wt_rows / np.maximum(
        np.linalg.norm(wt_rows, axis=1, keepdims=True), EPS)
    cos_t = np.einsum("bd,bd->b", xn64, wt_n)
    u = np.clip(cos_t, -1.0, 1.0)
    new_zy = u * COS_M - np.sqrt(np.maximum(0.0, 1.0 - u * u)) * SIN_M
    val = np.where(cos_t > 0.0, new_zy, cos_t)
    out[np.arange(B), target] = (S * val).astype(np.float32)
    return out


# revision 32
# speedup vs baseline: 1.2213x; 1.0122x over previous
"""ArcFace logits kernel for 8 Trainium2 NeuronCores.

out = (cos + one_hot_margin_body) * S  where cos = l2norm(x) @ l2norm(weight).T

Sharding: model-parallel over the class dim (12500 classes per core, padded to
12544).  x is replicated.  The host pre-normalizes both operands (folding the
S=64 scale into x), so each core is a pure bf16 matmul pipeline: uniform
1792-class macro tiles (12544 = 7 x 1792, so no ragged tail), [128,512]-psum
matmuls (K=512 over 4 k-tiles), PSUM->SBUF bf16 copy evacuation alternating
between the Scalar and Vector engines, and output DMA.  The ArcFace margin
touches only 1024 of the 102.4M outputs, so it is applied on the host (exact
f64) after gathering the shards.
"""

import math
import sys
import types

sys.path.insert(0, "/opt/trn_rl_repo")

import numpy as np
import ml_dtypes

# ---- register the NTFF profile hook that the container's antenv lacks ------
# (harmless if profiling is never requested; required for trace=True runs)
def _ensure_axon_hooks():
    try:
        import antenv
        if "antenv.axon_hooks" in sys.modules:
            return
        holder = {"h": None}
        mod = types.ModuleType("antenv.axon_hooks")
        mod.set_axon_ntff_profile_hook = lambda h: holder.__setitem__("h", h)
        mod.get_axon_ntff_profile_hook = lambda: holder["h"]
        sys.modules["antenv.axon_hooks"] = mod
        antenv.axon_hooks = mod
        try:
            from trn_agent_boot.trn_boot import _ntff_profile_via_ctypes
            mod.set_axon_ntff_profile_hook(
                _ntff_profile_via_ctypes("/opt/axon/libaxon_pjrt.so")
            )
        except Exception:
            pass
    except Exception:
        pass


_ensure_axon_hooks()

import concourse.bass as bass
import concourse.mybir as mybir
import concourse.tile as tile
from concourse import bacc
import concourse.bass_utils as bass_utils

bass_utils.upload_artifacts = lambda tmpdir: tmpdir  # no cloud in container

B = 1024
D = 512
C = 100000
NCORES = 8
CS = C // NCORES          # 12500 classes per core
CSP = 12544               # padded to 98 * 128
S = 64.0
ARC_M = 0.5
COS_M = math.cos(ARC_M)
SIN_M = math.sin(ARC_M)
EPS = 1e-12
MACRO = 1792              # classes per macro tile; 12544 = 7 x 1792 exactly
MACROS = [(i * MACRO, MACRO) for i in range(CSP // MACRO)]
DT = D // 128              # 4 contraction chunks
BT = B // 128              # 8 batch tiles

f32 = mybir.dt.float32
bf16 = mybir.dt.bfloat16

_CACHE = {}


def _build_graph(variant="v7"):
    nc = bacc.Bacc("TRN2", target_bir_lowering=False, debug=False,
                   num_devices=NCORES)

    xt_ext = nc.dram_tensor("xt", [D, B], bf16, kind="ExternalInput")
    wt_ext = nc.dram_tensor("wt", [D, CSP], bf16, kind="ExternalInput")
    out_ext = nc.dram_tensor("out", [B, CSP], bf16, kind="ExternalOutput")

    with tile.TileContext(nc) as tc:
        with (
            tc.tile_pool(name="persist", bufs=1) as persist,
            tc.tile_pool(name="wT", bufs=3) as wTp,
            tc.tile_pool(name="outsb", bufs=4) as outp,
            tc.tile_pool(name="psum_o", bufs=2, space="PSUM") as psum_op,
        ):
            # x comes pre-transposed / pre-normalized / pre-scaled (x64) bf16
            xnT = [persist.tile([128, B], bf16, tag=f"xnT{d}", name=f"xnT{d}")
                   for d in range(DT)]
            for d in range(DT):
                xq = nc.sync if variant == "v10" and d % 2 else nc.scalar
                xq.dma_start(out=xnT[d][:],
                             in_=xt_ext[d * 128:(d + 1) * 128, :])

            ei = 0
            for mi, (moff, mlen) in enumerate(MACROS):
                wT = [wTp.tile([128, MACRO], bf16, tag=f"wT{d}", name=f"wT{d}")
                      for d in range(DT)]
                for d in range(DT):
                    nc.gpsimd.dma_start(
                        out=wT[d][:, :mlen],
                        in_=wt_ext[d * 128:(d + 1) * 128, moff:moff + mlen])

                nss = [(i * 512, min(512, mlen - i * 512))
                       for i in range((mlen + 511) // 512)]
                for bt in range(BT):
                    po = [psum_op.tile([128, 512], f32, tag=f"po{i}",
                                       name=f"po{i}")
                          for i in range(len(nss))]
                    for i, (no, nw) in enumerate(nss):
                        for d in range(DT):
                            nc.tensor.matmul(
                                out=po[i][:, :nw],
                                lhsT=xnT[d][:, bt * 128:(bt + 1) * 128],
                                rhs=wT[d][:, no:no + nw],
                                start=(d == 0), stop=(d == DT - 1))

                    ob = outp.tile([128, MACRO], bf16, tag="ob")
                    for i, (no, nw) in enumerate(nss):
                        if ei % 2 == 0:
                            nc.scalar.copy(out=ob[:, no:no + nw],
                                           in_=po[i][:, :nw])
                        else:
                            nc.vector.tensor_scalar(
                                out=ob[:, no:no + nw], in0=po[i][:, :nw],
                                scalar1=1.0, scalar2=None,
                                op0=mybir.AluOpType.mult)
                        ei += 1

                    oq = nc.sync
                    if variant == "v11" and mi == len(MACROS) - 1 and bt % 2:
                        oq = nc.scalar
                    oq.dma_start(
                        out=out_ext[bt * 128:(bt + 1) * 128, moff:moff + mlen],
                        in_=ob[:, :mlen])

    nc.finalize()
    return nc


def _get_graph():
    import os
    variant = os.environ.get("K_VARIANT", "v7")
    if variant not in _CACHE:
        _CACHE[variant] = _build_graph(variant)
    return _CACHE[variant]


def kernel(x, weight, target):
    x = np.ascontiguousarray(np.asarray(x, dtype=np.float32))
    weight = np.ascontiguousarray(np.asarray(weight, dtype=np.float32))
    target = np.asarray(target).astype(np.int64)

    nc = _get_graph()

    xnorm = np.maximum(np.linalg.norm(x, axis=1, keepdims=True), EPS)
    xn = x / xnorm
    xt = np.ascontiguousarray((S * xn).T).astype(ml_dtypes.bfloat16)  # [D, B]

    wnorm = np.maximum(np.linalg.norm(weight, axis=1, keepdims=True), EPS)
    wn_t = (weight / wnorm).T  # [D, C] view
    in_maps = []
    for c in range(NCORES):
        c0 = c * CS
        wt = np.zeros((D, CSP), dtype=ml_dtypes.bfloat16)
        wt[:, :CS] = wn_t[:, c0:c0 + CS].astype(ml_dtypes.bfloat16)
        in_maps.append({"xt": xt, "wt": wt})

    from concourse.bass_utils import run_bass_kernel_spmd
    res = None
    last_err = None
    for attempt in range(3):
        try:
            res = run_bass_kernel_spmd(nc, in_maps, core_ids=list(range(NCORES)))
            break
        except Exception as e:  # transient NRT_EXEC_UNIT_UNRECOVERABLE flakes
            last_err = e
            import time as _time
            _time.sleep(5)
    if res is None:
        raise last_err

    out = np.concatenate(
        [res.results[c]["out"][:, :CS].astype(np.float32) for c in range(NCORES)],
        axis=1)

    # ArcFace margin for the 1024 (row, target) entries, exact on host
    xn64 = x.astype(np.float64) / np.maximum(
        np.linalg.norm(x.astype(np.float64), axis=1, keepdims=True), EPS)
    wt_rows = weight[target].astype(np.float64)
    wt_n = wt_rows / np.maximum(
        np.linalg.norm(wt_rows, axis=1, keepdims=True), EPS)
    cos_t = np.einsum("bd,bd->b", xn64, wt_n)
    u = np.clip(cos_t, -1.0, 1.0)
    new_zy = u * COS_M - np.sqrt(np.maximum(0.0, 1.0 - u * u)) * SIN_M
    val = np.where(cos_t > 0.0, new_zy, cos_t)
    out[np.arange(B), target] = (S * val).astype(np.float32)
    return out
